# revision 58
# baseline (speedup 1.0000x reference)
"""Multi-head causal attention (B=4, T=2048, H=16, D=64) on 8 trn2 NeuronCores.

Sharding: core c = (batch b = c//2, head-group hg = c%2 of 8 heads).
Each core computes its batch's QKV projection for its 8 heads, causal
attention, and a partial output projection (contraction over its 512
channels of W_proj). Host sums the two partials per batch and adds bias.

Schedule: the QKV / output-projection GEMM matmuls are queued as
"filler" steps and dripped between attention score/AV ops so the PE
stays busy while the scalar engine computes the softmax exps (exp
throughput ~956ns per [128,2,512] tile vs ~540ns of attention matmul
per k-tile).  Tag-ordered flushes force each GEMM chain out just
before its first consumer.

Layouts (host-side repacked so every DMA line is 8KB contiguous):
  - xh   [128(ci), 4(tb), 8(co), 512(t)]   x^T tiled
  - wk/wq/wv [128(ci), 8(co), 512(m)]
  - wp   [128(ci), 4(co), 1024(n)]
  - y    [128(p), 16(tt), 1024(n)] fp16 output (partials summed on host)

Per-core kernel layout (as v1):
  - K^T, Q^T stored [hd, t]: head-dim on partitions, 2 heads per tile.
  - V stored [t, h*65+d] with ones column -> AV matmul emits softmax
    denominators in row 64 for free.
  - Scores computed transposed S_T[k, q]; P_T = exp(S_T) is the AV
    moving operand; no max subtraction needed (|scores/8| small).
"""

import os
import sys

import numpy as np

F16_NP = np.dtype(np.float16)

if "/opt/trn_rl_repo" not in sys.path:
    sys.path.insert(0, "/opt/trn_rl_repo")

from collections import deque
from contextlib import ExitStack

import concourse.bass as bass
import concourse.bacc as bacc
import concourse.mybir as mybir
import concourse.tile as tile
from concourse._compat import with_exitstack

P = 128
T = 2048
C = 1024
H_PER_CORE = 8
D = 64
DP = D + 1  # V augmented with a ones column
NC_CORES = 8

TB = 4  # t-blocks of 512
QB = 4  # q-blocks of 512
CI = 8  # contraction tiles of 128 over C for QKV proj

F32 = mybir.dt.float32
F16 = mybir.dt.float16  # full matmul rate, 8x finer mantissa than bf16


@with_exitstack
def build_attention_kernel(ctx: ExitStack, tc: tile.TileContext):
    nc = tc.nc

    xh = nc.declare_dram_parameter("xh", [P, TB, CI, 512], F16, isOutput=False)
    wk = nc.declare_dram_parameter("wk", [P, CI, 512], F16, isOutput=False)
    wq = nc.declare_dram_parameter("wq", [P, CI, 512], F16, isOutput=False)
    wv = nc.declare_dram_parameter("wv", [P, CI, 512], F16, isOutput=False)
    wp = nc.declare_dram_parameter("wp", [P, 4, C], F16, isOutput=False)
    y = nc.declare_dram_parameter("y", [P, 16, C], F16, isOutput=True)

    # ---- SBUF pools ----
    kt_pool = ctx.enter_context(tc.tile_pool(name="ktp", bufs=16))
    qt_pool = ctx.enter_context(tc.tile_pool(name="qtp", bufs=16))
    ot_pool = ctx.enter_context(tc.tile_pool(name="otp", bufs=16))
    v_pool = ctx.enter_context(tc.tile_pool(name="vp", bufs=4))
    const_pool = ctx.enter_context(tc.tile_pool(name="constp", bufs=1))
    w_pool = ctx.enter_context(tc.tile_pool(name="wp_", bufs=1))
    xt_pool = ctx.enter_context(tc.tile_pool(name="xtp", bufs=4))
    pt_pool = ctx.enter_context(tc.tile_pool(name="ptp", bufs=8))
    recip_pool = ctx.enter_context(tc.tile_pool(name="recipp", bufs=4))
    bc_pool = ctx.enter_context(tc.tile_pool(name="bcp", bufs=4))
    oraw_pool = ctx.enter_context(tc.tile_pool(name="orawp", bufs=4))
    y_pool = ctx.enter_context(tc.tile_pool(name="yp", bufs=2))
    # ---- PSUM: 4 banks scores + 2 banks AV + 2 banks GEMM filler = 8 ----
    ps_s_pool = ctx.enter_context(tc.tile_pool(name="ps_s", bufs=2, space="PSUM"))
    ps_o_pool = ctx.enter_context(tc.tile_pool(name="ps_o", bufs=2, space="PSUM"))
    ps_f_pool = ctx.enter_context(tc.tile_pool(name="ps_f", bufs=2, space="PSUM"))

    # KT[pt][tb], QT[pt][qb]: [128, 512]; partitions = 2 heads x 64 dims
    KT = [[kt_pool.tile([P, 512], F16, tag="kt", name=f"KT_{pt}_{tb}") for tb in range(TB)] for pt in range(4)]
    QT = [[qt_pool.tile([P, 512], F16, tag="qt", name=f"QT_{pt}_{qb}") for qb in range(QB)] for pt in range(4)]
    OT = [[ot_pool.tile([P, 512], F16, tag="ot", name=f"OT_{hp}_{qb}") for qb in range(QB)] for hp in range(4)]
    V = [v_pool.tile([P, 4, H_PER_CORE * DP], F16, tag="v", name=f"V_{tb}") for tb in range(TB)]
    masks = const_pool.tile([P, 4, 512], F16, tag="masks", name="masks")
    wk_sb = w_pool.tile([P, CI, 512], F16)
    wq_sb = w_pool.tile([P, CI, 512], F16)
    wv_sb = w_pool.tile([P, CI, 512], F16)
    wp_sb = w_pool.tile([P, 4, C], F16)
    xts = [xt_pool.tile([P, CI, 512], F16, tag="xt", name=f"xt_{tb}") for tb in range(TB)]

    # diagonal causal masks: masks[:, j, :][kk, qq] = 1.0 if qq >= kk + j*128
    for j in range(4):
        nc.gpsimd.memset(masks[:, j, :], 1.0)
        nc.gpsimd.affine_select(
            out=masks[:, j, :],
            in_=masks[:, j, :],
            compare_op=mybir.AluOpType.is_ge,
            fill=0.0,
            base=-j * P,
            pattern=[[1, 512]],
            channel_multiplier=-1,
        )
    # ones column of V
    for tb in range(TB):
        ones_col = V[tb].rearrange("p s (h e) -> p s h e", e=DP)[:, :, :, D : D + 1]
        nc.gpsimd.memset(ones_col, 1.0)

    # ---- DMAs: wq / x chunks first (Q chains run first) so the PE starts
    # ASAP; then prefetch everything (all lines 8KB contiguous). ----
    nc.sync.dma_start(wq_sb[:, 0:1], wq[:, 0:1])
    nc.sync.dma_start(xts[0][:, 0:1], xh[:, 0, 0:1])
    nc.sync.dma_start(wq_sb[:, 1:2], wq[:, 1:2])
    nc.sync.dma_start(xts[0][:, 1:2], xh[:, 0, 1:2])
    nc.sync.dma_start(wq_sb[:, 2:3], wq[:, 2:3])
    nc.sync.dma_start(xts[0][:, 2:3], xh[:, 0, 2:3])
    nc.sync.dma_start(wq_sb[:, 3:], wq[:, 3:])
    nc.sync.dma_start(xts[0][:, 3:5], xh[:, 0, 3:5])
    nc.sync.dma_start(xts[0][:, 5:8], xh[:, 0, 5:8])
    nc.sync.dma_start(wk_sb[:], wk[:])
    nc.sync.dma_start(wv_sb[:], wv[:])
    nc.sync.dma_start(xts[1][:], xh[:, 1])
    nc.sync.dma_start(xts[2][:], xh[:, 2])
    nc.sync.dma_start(xts[3][:], xh[:, 3])
    nc.sync.dma_start(wp_sb[:], wp[:])

    # ================= filler machinery =================
    # Each filler item: (tag, fn). Tags are appended nondecreasing.
    # qkv chain tags: tb*1000 + {Q0..Q3: 0..3, K0..K3: 4..7, V0..V3: 8..11}
    # (Q flushes at pair start; K lazily at the first diagonal score; V at
    # the diagonal AVs — spreads forced chains across each pair.)
    # proj tags: 10000 + qb*10
    filler = deque()
    drip_clock = [0]

    def drip():
        # 2,2,1 pattern ~= the per-kt PE deficit (exp 956ns vs ~540ns of
        # attention matmul); rations filler so it lasts to the final pair
        n = 1 if drip_clock[0] % 3 == 2 else 2
        drip_clock[0] += 1
        while n > 0 and filler:
            _, fn = filler.popleft()
            fn()
            n -= 1

    def flush_until(tag_limit):
        while filler and filler[0][0] <= tag_limit:
            _, fn = filler.popleft()
            fn()

    def flush_all():
        while filler:
            _, fn = filler.popleft()
            fn()

    def add_kq_chain(tb, kind, pt):
        """K^T / Q^T chain: out[hd, t] for 128 hd (2 heads), 512 t."""
        w_sb = wk_sb if kind == "K" else wq_sb
        dst = KT if kind == "K" else QT
        tag = tb * 1000 + pt + (4 if kind == "K" else 0)
        ps_ref = []

        def mk(ci):
            def fn():
                if ci == 0:
                    ps_ref.append(ps_f_pool.tile([P, 512], F32, tag="f_ps", name="f_ps"))
                nc.tensor.matmul(
                    ps_ref[0][:],
                    lhsT=w_sb[:, ci, pt * P : (pt + 1) * P],
                    rhs=xts[tb][:, ci, :],
                    start=(ci == 0),
                    stop=(ci == CI - 1),
                )
                if ci == CI - 1:
                    nc.vector.tensor_copy(dst[pt][tb][:], ps_ref[0][:])

            return fn

        for ci in range(CI):
            filler.append((tag, mk(ci)))

    def add_v_chain(tb, ts):
        """V chain: out[t-slice 128, h*d 512] scattered into V[tb] layout."""
        tag = tb * 1000 + 8 + ts
        ps_ref = []

        def mk(ci):
            def fn():
                if ci == 0:
                    ps_ref.append(ps_f_pool.tile([P, 512], F32, tag="f_ps", name="f_ps"))
                nc.tensor.matmul(
                    ps_ref[0][:],
                    lhsT=xts[tb][:, ci, ts * P : (ts + 1) * P],
                    rhs=wv_sb[:, ci, :],
                    start=(ci == 0),
                    stop=(ci == CI - 1),
                )
                if ci == CI - 1:
                    nc.vector.tensor_copy(
                        V[tb][:, ts].rearrange("p (h e) -> p h e", e=DP)[:, :, :D],
                        ps_ref[0][:].rearrange("p (h d) -> p h d", d=D),
                    )

            return fn

        for ci in range(CI):
            filler.append((tag, mk(ci)))

    ysbs = {}

    def add_proj_chain(qb, tt, nb):
        """Output projection partial: y[t-tile, 512 nb-cols] = sum_ct OT."""
        tag = 10000 + qb * 10
        sub = tt % 4
        ps_ref = []

        def mk(ct):
            def fn():
                if ct == 0:
                    ps_ref.append(ps_f_pool.tile([P, 512], F32, tag="f_ps", name="f_ps"))
                nc.tensor.matmul(
                    ps_ref[0][:],
                    lhsT=OT[ct][qb][:, sub * P : (sub + 1) * P],
                    rhs=wp_sb[:, ct, nb * 512 : (nb + 1) * 512],
                    start=(ct == 0),
                    stop=(ct == 3),
                )
                if ct == 3:
                    if tt % 2 == 0 and nb == 0:
                        ysbs[tt // 2] = y_pool.tile([P, 2, C], F16, tag="ypair", name="ypair")
                    ysb = ysbs[tt // 2]
                    nc.vector.tensor_copy(
                        ysb[:, tt % 2, nb * 512 : (nb + 1) * 512], ps_ref[0][:]
                    )
                    if tt % 2 == 1 and nb == 1:
                        nc.sync.dma_start(y[:, tt - 1 : tt + 1, :], ysb[:])

            return fn

        for ct in range(4):
            filler.append((tag, mk(ct)))

    for tb in range(TB):
        for pt in range(4):
            add_kq_chain(tb, "Q", pt)
        for pt in range(4):
            add_kq_chain(tb, "K", pt)
        for ts in range(4):
            add_v_chain(tb, ts)

    # ================= attention =================
    pending_normalize = []

    def attention_pair(qb, hp):
        ot_ps = [ps_o_pool.tile([DP, 512], F32, tag="ot_ps", name=f"ot_ps_{i}") for i in range(2)]
        nkt = 4 * (qb + 1)
        pts = {}

        def emit_scores_exp(kt):
            tb = kt // 4
            if kt >= 4 * qb:  # diagonal score needs this qb's K chain
                flush_until(qb * 1000 + 4 + hp)
            # diagonal tiles: only q >= j*128 is (partially) visible
            qs = (kt - 4 * qb) * P if kt >= 4 * qb else 0
            nq = 512 - qs
            s_ps = ps_s_pool.tile([P, 2, 512], F32, tag="s_ps", name="s_ps")
            for h2 in range(2):
                # S_T[k, q] for head h = 2*hp + h2 (row-packed pair)
                nc.tensor.matmul(
                    s_ps[:, h2, qs:],
                    lhsT=KT[hp][tb][
                        h2 * D : (h2 + 1) * D,
                        (kt % 4) * P : (kt % 4 + 1) * P,
                    ],
                    rhs=QT[hp][qb][h2 * D : (h2 + 1) * D, qs:],
                    start=True,
                    stop=True,
                )
            p_t = pt_pool.tile([P, 2, 512], F16, tag="pt", name="p_t")
            nc.scalar.activation(
                p_t[:, :, qs:],
                s_ps[:, :, qs:],
                mybir.ActivationFunctionType.Exp,
                scale=0.125,
            )
            if kt >= 4 * qb:  # diagonal: zero q < k entries.  Only the first
                # 128 q-columns of the tile can be masked (q >= k holds for
                # all k once q passes the k-tile) -> 1/4 the mask-mul work.
                j = kt - 4 * qb
                mb = masks[:, j : j + 1, qs : qs + P].to_broadcast([P, 2, P])
                nc.vector.tensor_mul(
                    p_t[:, :, qs : qs + P], p_t[:, :, qs : qs + P], mb
                )
            pts[kt] = (p_t, qs)

        def emit_av(kt):
            tb = kt // 4
            if kt >= 4 * qb:  # diagonal AV needs V[qb] chain ts = kt-4qb
                flush_until(qb * 1000 + 8 + (kt - 4 * qb))
            p_t, qs = pts.pop(kt)
            for h2 in range(2):
                h = 2 * hp + h2
                nc.tensor.matmul(
                    ot_ps[h2][:, qs:],
                    lhsT=V[tb][:, kt % 4, h * DP : (h + 1) * DP],
                    rhs=p_t[:, h2, qs:],
                    start=(kt == 0),
                    stop=(kt == nkt - 1),
                )

        # software pipeline: S(kt+1) before AV(kt); drip GEMM filler so the
        # PE keeps busy while ACT digests the exps
        emit_scores_exp(0)
        for kt in range(1, nkt):
            emit_scores_exp(kt)
            emit_av(kt - 1)
            drip()
            # emit the previous pair's deferred normalize: for qb>0 the first
            # kts are off-diagonal (no mask-muls on vector to delay), so pop
            # early; for qb==0 every kt is diagonal, pop after those masks
            if kt == (3 if qb == 0 else 1) and pending_normalize:
                pending_normalize.pop()()
        emit_av(nkt - 1)

        # free the AV PSUM banks ASAP with one fast copy each (the next
        # pair's first AV WARs on these banks); normalize runs from SBUF
        # later, off every critical path.  fp16 staging: 2x DVE rate, and
        # raw |O| <~1e4, denom <~3e4 fit fp16 comfortably.
        oraws = []
        for h2 in range(2):
            o_raw = oraw_pool.tile([DP, 512], F16, tag="oraw", name="o_raw")
            eng = nc.scalar.copy if h2 == 0 else nc.vector.tensor_copy
            eng(o_raw[:], ot_ps[h2][:])
            oraws.append(o_raw)

        def normalize():
            # divide rows 0..63 by the sums row (64); both heads' denominator
            # vectors batched into one recip / cast / broadcast
            recip = recip_pool.tile([1, 2, 512], F32, tag="recip", name="recip")
            for h2 in range(2):
                nc.vector.tensor_copy(recip[:, h2, :], oraws[h2][D : D + 1, :])
            nc.vector.reciprocal_approx_fast(recip[:], recip[:])
            bc16 = recip_pool.tile([1, 2, 512], F16, tag="recip16", name="recip16")
            nc.vector.tensor_copy(bc16[:], recip[:])
            bc = bc_pool.tile([D, 2, 512], F16, tag="bc", name="bc")
            nc.gpsimd.partition_broadcast(bc[:], bc16[:])
            for h2 in range(2):
                nc.vector.tensor_mul(
                    OT[hp][qb][h2 * D : (h2 + 1) * D, :],
                    oraws[h2][:D, :],
                    bc[:, h2, :],
                )

        pending_normalize.append(normalize)

    def proj_tile_direct(tt):
        """Tail projection using the (now free) scores PSUM banks."""
        qb, sub = tt // 4, tt % 4
        y_pair_ps = ps_s_pool.tile([P, 2, 512], F32, tag="s_ps", name="y_ps")
        for ct in range(4):
            lhsT = OT[ct][qb][:, sub * P : (sub + 1) * P]
            for nb in range(2):
                nc.tensor.matmul(
                    y_pair_ps[:, nb, :],
                    lhsT=lhsT,
                    rhs=wp_sb[:, ct, nb * 512 : (nb + 1) * 512],
                    start=(ct == 0),
                    stop=(ct == 3),
                )
        if tt % 2 == 0:
            ysbs[tt // 2] = y_pool.tile([P, 2, C], F16, tag="ypair", name="ypair")
        ysb = ysbs[tt // 2]
        for nb in range(2):
            # split engines so the two copies run in parallel at the tail
            eng = nc.scalar.copy if nb == 0 else nc.vector.tensor_copy
            eng(ysb[:, tt % 2, nb * 512 : (nb + 1) * 512], y_pair_ps[:, nb, :])
        if tt >= 14:  # last tiles: DMA singly so the writes start earlier
            nc.sync.dma_start(y[:, tt : tt + 1, :], ysb[:, tt % 2 : tt % 2 + 1, :])
        elif tt % 2 == 1:
            nc.sync.dma_start(y[:, tt - 1 : tt + 1, :], ysb[:])

    # ================= main schedule =================
    flush_until(7)  # all tb=0 K,Q chains: queue PE work spanning DMA arrival
    for qb in range(QB):
        for hp in range(4):
            flush_until(qb * 1000 + hp)  # Q chain for this pair
            attention_pair(qb, hp)
        if qb < 3:
            for tt in range(4 * qb, 4 * qb + 4):
                for nb in range(2):
                    add_proj_chain(qb, tt, nb)
    while pending_normalize:
        pending_normalize.pop()()
    flush_all()
    for tt in range(12, 16):
        proj_tile_direct(tt)

    return nc


_CACHED_NC = None


def get_nc():
    global _CACHED_NC
    if _CACHED_NC is None:
        nc = bacc.Bacc()
        with tile.TileContext(nc) as tc:
            build_attention_kernel(tc)
        nc.compile()
        _CACHED_NC = nc
    return _CACHED_NC


def make_in_maps(x, W_att, W_proj):
    x = np.asarray(x, dtype=np.float32)
    W_att = np.asarray(W_att, dtype=np.float32)
    W_proj = np.asarray(W_proj, dtype=np.float32)
    in_maps = []
    for c in range(NC_CORES):
        b, hg = c // 2, c % 2
        s = hg * 512
        # xh[ci, tb, co, t'] = x[b][tb*512+t', co*128+ci]
        xh = np.ascontiguousarray(
            x[b].reshape(TB, 512, CI, P).transpose(3, 0, 2, 1)
        ).astype(F16_NP)

        def wslice(w):
            # [128(ci), 8(co), 512(m)]
            return np.ascontiguousarray(
                w.reshape(CI, P, 512).transpose(1, 0, 2)
            ).astype(F16_NP)

        in_maps.append(
            {
                "xh": xh,
                "wk": wslice(W_att[:, 0 * C + s : 0 * C + s + 512]),
                "wq": wslice(W_att[:, 1 * C + s : 1 * C + s + 512]),
                "wv": wslice(W_att[:, 2 * C + s : 2 * C + s + 512]),
                "wp": np.ascontiguousarray(
                    W_proj[s : s + 512].reshape(4, P, C).transpose(1, 0, 2)
                ).astype(F16_NP),
            }
        )
    return in_maps


def combine_outputs(results, b_proj):
    B = NC_CORES // 2
    out = np.empty((B, T, C), dtype=np.float32)
    bias = np.asarray(b_proj, dtype=np.float32)
    for b in range(B):
        # y [128(p), 16(tt), 1024] -> [T, C] with t = tt*128 + p
        y0 = np.moveaxis(results[2 * b]["y"].astype(np.float32), 0, 1).reshape(T, C)
        y1 = np.moveaxis(results[2 * b + 1]["y"].astype(np.float32), 0, 1).reshape(T, C)
        out[b] = y0 + y1 + bias
    return out


def kernel(x, W_att, W_proj, b_proj):
    from concourse.bass_utils import run_bass_kernel_spmd

    nc = get_nc()
    in_maps = make_in_maps(x, W_att, W_proj)
    res = run_bass_kernel_spmd(nc, in_maps, list(range(NC_CORES)))
    return combine_outputs(res.results, b_proj)


# revision 60
# speedup vs baseline: 1.0031x; 1.0031x over previous
"""Multi-head causal attention (B=4, T=2048, H=16, D=64) on 8 trn2 NeuronCores.

Sharding: core c = (batch b = c//2, head-group hg = c%2 of 8 heads).
Each core computes its batch's QKV projection for its 8 heads, causal
attention, and a partial output projection (contraction over its 512
channels of W_proj). Host sums the two partials per batch and adds bias.

Schedule: the QKV / output-projection GEMM matmuls are queued as
"filler" steps and dripped between attention score/AV ops so the PE
stays busy while the scalar engine computes the softmax exps (exp
throughput ~956ns per [128,2,512] tile vs ~540ns of attention matmul
per k-tile).  Tag-ordered flushes force each GEMM chain out just
before its first consumer.

Layouts (host-side repacked so every DMA line is 8KB contiguous):
  - xh   [128(ci), 4(tb), 8(co), 512(t)]   x^T tiled
  - wk/wq/wv [128(ci), 8(co), 512(m)]
  - wp   [128(ci), 4(co), 1024(n)]
  - y    [128(p), 16(tt), 1024(n)] fp16 output (partials summed on host)

Per-core kernel layout (as v1):
  - K^T, Q^T stored [hd, t]: head-dim on partitions, 2 heads per tile.
  - V stored [t, h*65+d] with ones column -> AV matmul emits softmax
    denominators in row 64 for free.
  - Scores computed transposed S_T[k, q]; P_T = exp(S_T) is the AV
    moving operand; no max subtraction needed (|scores/8| small).
"""

import os
import sys

import numpy as np

F16_NP = np.dtype(np.float16)

if "/opt/trn_rl_repo" not in sys.path:
    sys.path.insert(0, "/opt/trn_rl_repo")

from collections import deque
from contextlib import ExitStack

import concourse.bass as bass
import concourse.bacc as bacc
import concourse.mybir as mybir
import concourse.tile as tile
from concourse._compat import with_exitstack

P = 128
T = 2048
C = 1024
H_PER_CORE = 8
D = 64
DP = D + 1  # V augmented with a ones column
NC_CORES = 8

TB = 4  # t-blocks of 512
QB = 4  # q-blocks of 512
CI = 8  # contraction tiles of 128 over C for QKV proj

F32 = mybir.dt.float32
F16 = mybir.dt.float16  # full matmul rate, 8x finer mantissa than bf16


@with_exitstack
def build_attention_kernel(ctx: ExitStack, tc: tile.TileContext):
    nc = tc.nc

    xh = nc.declare_dram_parameter("xh", [P, TB, CI, 512], F16, isOutput=False)
    wk = nc.declare_dram_parameter("wk", [P, CI, 512], F16, isOutput=False)
    wq = nc.declare_dram_parameter("wq", [P, CI, 512], F16, isOutput=False)
    wv = nc.declare_dram_parameter("wv", [P, CI, 512], F16, isOutput=False)
    wp = nc.declare_dram_parameter("wp", [P, 4, C], F16, isOutput=False)
    y = nc.declare_dram_parameter("y", [P, 16, C], F16, isOutput=True)

    # ---- SBUF pools ----
    kt_pool = ctx.enter_context(tc.tile_pool(name="ktp", bufs=16))
    qt_pool = ctx.enter_context(tc.tile_pool(name="qtp", bufs=16))
    ot_pool = ctx.enter_context(tc.tile_pool(name="otp", bufs=16))
    v_pool = ctx.enter_context(tc.tile_pool(name="vp", bufs=4))
    const_pool = ctx.enter_context(tc.tile_pool(name="constp", bufs=1))
    w_pool = ctx.enter_context(tc.tile_pool(name="wp_", bufs=1))
    xt_pool = ctx.enter_context(tc.tile_pool(name="xtp", bufs=4))
    pt_pool = ctx.enter_context(tc.tile_pool(name="ptp", bufs=8))
    recip_pool = ctx.enter_context(tc.tile_pool(name="recipp", bufs=4))
    bc_pool = ctx.enter_context(tc.tile_pool(name="bcp", bufs=4))
    oraw_pool = ctx.enter_context(tc.tile_pool(name="orawp", bufs=4))
    y_pool = ctx.enter_context(tc.tile_pool(name="yp", bufs=2))
    # ---- PSUM: 4 banks scores + 2 banks AV + 2 banks GEMM filler = 8 ----
    ps_s_pool = ctx.enter_context(tc.tile_pool(name="ps_s", bufs=2, space="PSUM"))
    ps_o_pool = ctx.enter_context(tc.tile_pool(name="ps_o", bufs=2, space="PSUM"))
    ps_f_pool = ctx.enter_context(tc.tile_pool(name="ps_f", bufs=2, space="PSUM"))

    # KT[pt][tb], QT[pt][qb]: [128, 512]; partitions = 2 heads x 64 dims
    KT = [[kt_pool.tile([P, 512], F16, tag="kt", name=f"KT_{pt}_{tb}") for tb in range(TB)] for pt in range(4)]
    QT = [[qt_pool.tile([P, 512], F16, tag="qt", name=f"QT_{pt}_{qb}") for qb in range(QB)] for pt in range(4)]
    OT = [[ot_pool.tile([P, 512], F16, tag="ot", name=f"OT_{hp}_{qb}") for qb in range(QB)] for hp in range(4)]
    V = [v_pool.tile([P, 4, H_PER_CORE * DP], F16, tag="v", name=f"V_{tb}") for tb in range(TB)]
    masks = const_pool.tile([P, 4, 512], F16, tag="masks", name="masks")
    wk_sb = w_pool.tile([P, CI, 512], F16)
    wq_sb = w_pool.tile([P, CI, 512], F16)
    wv_sb = w_pool.tile([P, CI, 512], F16)
    wp_sb = w_pool.tile([P, 4, C], F16)
    xts = [xt_pool.tile([P, CI, 512], F16, tag="xt", name=f"xt_{tb}") for tb in range(TB)]

    # diagonal causal masks: masks[:, j, :][kk, qq] = 1.0 if qq >= kk + j*128
    for j in range(4):
        nc.gpsimd.memset(masks[:, j, :], 1.0)
        nc.gpsimd.affine_select(
            out=masks[:, j, :],
            in_=masks[:, j, :],
            compare_op=mybir.AluOpType.is_ge,
            fill=0.0,
            base=-j * P,
            pattern=[[1, 512]],
            channel_multiplier=-1,
        )
    # ones column of V
    for tb in range(TB):
        ones_col = V[tb].rearrange("p s (h e) -> p s h e", e=DP)[:, :, :, D : D + 1]
        nc.gpsimd.memset(ones_col, 1.0)

    # ---- DMAs: wq / x chunks first (Q chains run first) so the PE starts
    # ASAP; then prefetch everything (all lines 8KB contiguous). ----
    nc.sync.dma_start(wq_sb[:, 0:1], wq[:, 0:1])
    nc.sync.dma_start(xts[0][:, 0:1], xh[:, 0, 0:1])
    nc.sync.dma_start(wq_sb[:, 1:2], wq[:, 1:2])
    nc.sync.dma_start(xts[0][:, 1:2], xh[:, 0, 1:2])
    nc.sync.dma_start(wq_sb[:, 2:3], wq[:, 2:3])
    nc.sync.dma_start(xts[0][:, 2:3], xh[:, 0, 2:3])
    nc.sync.dma_start(wq_sb[:, 3:], wq[:, 3:])
    nc.sync.dma_start(xts[0][:, 3:5], xh[:, 0, 3:5])
    nc.sync.dma_start(xts[0][:, 5:8], xh[:, 0, 5:8])
    nc.sync.dma_start(wk_sb[:], wk[:])
    nc.sync.dma_start(wv_sb[:], wv[:])
    nc.sync.dma_start(xts[1][:], xh[:, 1])
    nc.sync.dma_start(xts[2][:], xh[:, 2])
    nc.sync.dma_start(xts[3][:], xh[:, 3])
    nc.sync.dma_start(wp_sb[:], wp[:])

    # ================= filler machinery =================
    # Each filler item: (tag, fn). Tags are appended nondecreasing.
    # qkv chain tags: tb*1000 + {Q0..Q3: 0..3, K0..K3: 4..7, V0..V3: 8..11}
    # (Q flushes at pair start; K lazily at the first diagonal score; V at
    # the diagonal AVs — spreads forced chains across each pair.)
    # proj tags: 10000 + qb*10
    filler = deque()
    drip_clock = [0]

    def drip():
        # 2,2,1 pattern ~= the per-kt PE deficit (exp 956ns vs ~540ns of
        # attention matmul); rations filler so it lasts to the final pair
        n = 1 if drip_clock[0] % 3 == 2 else 2
        drip_clock[0] += 1
        while n > 0 and filler:
            _, fn = filler.popleft()
            fn()
            n -= 1

    def flush_until(tag_limit):
        while filler and filler[0][0] <= tag_limit:
            _, fn = filler.popleft()
            fn()

    def flush_all():
        while filler:
            _, fn = filler.popleft()
            fn()

    def add_kq_chain(tb, kind, pt):
        """K^T / Q^T chain: out[hd, t] for 128 hd (2 heads), 512 t."""
        w_sb = wk_sb if kind == "K" else wq_sb
        dst = KT if kind == "K" else QT
        tag = tb * 1000 + pt + (4 if kind == "K" else 0)
        ps_ref = []

        def mk(ci):
            def fn():
                if ci == 0:
                    ps_ref.append(ps_f_pool.tile([P, 512], F32, tag="f_ps", name="f_ps"))
                nc.tensor.matmul(
                    ps_ref[0][:],
                    lhsT=w_sb[:, ci, pt * P : (pt + 1) * P],
                    rhs=xts[tb][:, ci, :],
                    start=(ci == 0),
                    stop=(ci == CI - 1),
                )
                if ci == CI - 1:
                    nc.vector.tensor_copy(dst[pt][tb][:], ps_ref[0][:])

            return fn

        for ci in range(CI):
            filler.append((tag, mk(ci)))

    def add_v_chain(tb, ts):
        """V chain: out[t-slice 128, h*d 512] scattered into V[tb] layout."""
        tag = tb * 1000 + 8 + ts
        ps_ref = []

        def mk(ci):
            def fn():
                if ci == 0:
                    ps_ref.append(ps_f_pool.tile([P, 512], F32, tag="f_ps", name="f_ps"))
                nc.tensor.matmul(
                    ps_ref[0][:],
                    lhsT=xts[tb][:, ci, ts * P : (ts + 1) * P],
                    rhs=wv_sb[:, ci, :],
                    start=(ci == 0),
                    stop=(ci == CI - 1),
                )
                if ci == CI - 1:
                    nc.vector.tensor_copy(
                        V[tb][:, ts].rearrange("p (h e) -> p h e", e=DP)[:, :, :D],
                        ps_ref[0][:].rearrange("p (h d) -> p h d", d=D),
                    )

            return fn

        for ci in range(CI):
            filler.append((tag, mk(ci)))

    ysbs = {}

    def add_proj_chain(qb, tt, nb):
        """Output projection partial: y[t-tile, 512 nb-cols] = sum_ct OT."""
        tag = 10000 + qb * 10
        sub = tt % 4
        ps_ref = []

        def mk(ct):
            def fn():
                if ct == 0:
                    ps_ref.append(ps_f_pool.tile([P, 512], F32, tag="f_ps", name="f_ps"))
                nc.tensor.matmul(
                    ps_ref[0][:],
                    lhsT=OT[ct][qb][:, sub * P : (sub + 1) * P],
                    rhs=wp_sb[:, ct, nb * 512 : (nb + 1) * 512],
                    start=(ct == 0),
                    stop=(ct == 3),
                )
                if ct == 3:
                    if tt % 2 == 0 and nb == 0:
                        ysbs[tt // 2] = y_pool.tile([P, 2, C], F16, tag="ypair", name="ypair")
                    ysb = ysbs[tt // 2]
                    nc.vector.tensor_copy(
                        ysb[:, tt % 2, nb * 512 : (nb + 1) * 512], ps_ref[0][:]
                    )
                    if tt % 2 == 1 and nb == 1:
                        nc.sync.dma_start(y[:, tt - 1 : tt + 1, :], ysb[:])

            return fn

        for ct in range(4):
            filler.append((tag, mk(ct)))

    for tb in range(TB):
        for pt in range(4):
            add_kq_chain(tb, "Q", pt)
        for pt in range(4):
            add_kq_chain(tb, "K", pt)
        for ts in range(4):
            add_v_chain(tb, ts)

    # ================= attention =================
    pending_normalize = []

    def attention_pair(qb, hp):
        ot_ps = [ps_o_pool.tile([DP, 512], F32, tag="ot_ps", name=f"ot_ps_{i}") for i in range(2)]
        nkt = 4 * (qb + 1)
        pts = {}

        def emit_scores_exp(kt):
            tb = kt // 4
            if kt >= 4 * qb:  # diagonal score needs this qb's K chain
                flush_until(qb * 1000 + 4 + hp)
            # diagonal tiles: only q >= j*128 is (partially) visible
            qs = (kt - 4 * qb) * P if kt >= 4 * qb else 0
            nq = 512 - qs
            s_ps = ps_s_pool.tile([P, 2, 512], F32, tag="s_ps", name="s_ps")
            for h2 in range(2):
                # S_T[k, q] for head h = 2*hp + h2 (row-packed pair)
                nc.tensor.matmul(
                    s_ps[:, h2, qs:],
                    lhsT=KT[hp][tb][
                        h2 * D : (h2 + 1) * D,
                        (kt % 4) * P : (kt % 4 + 1) * P,
                    ],
                    rhs=QT[hp][qb][h2 * D : (h2 + 1) * D, qs:],
                    start=True,
                    stop=True,
                )
            p_t = pt_pool.tile([P, 2, 512], F16, tag="pt", name="p_t")
            nc.scalar.activation(
                p_t[:, :, qs:],
                s_ps[:, :, qs:],
                mybir.ActivationFunctionType.Exp,
                scale=0.125,
            )
            if kt >= 4 * qb:  # diagonal: zero q < k entries.  Only the first
                # 128 q-columns of the tile can be masked (q >= k holds for
                # all k once q passes the k-tile) -> 1/4 the mask-mul work.
                j = kt - 4 * qb
                mb = masks[:, j : j + 1, qs : qs + P].to_broadcast([P, 2, P])
                nc.vector.tensor_mul(
                    p_t[:, :, qs : qs + P], p_t[:, :, qs : qs + P], mb
                )
            pts[kt] = (p_t, qs)

        def emit_av(kt):
            tb = kt // 4
            if kt >= 4 * qb:  # diagonal AV needs V[qb] chain ts = kt-4qb
                flush_until(qb * 1000 + 8 + (kt - 4 * qb))
            p_t, qs = pts.pop(kt)
            for h2 in range(2):
                h = 2 * hp + h2
                nc.tensor.matmul(
                    ot_ps[h2][:, qs:],
                    lhsT=V[tb][:, kt % 4, h * DP : (h + 1) * DP],
                    rhs=p_t[:, h2, qs:],
                    start=(kt == 0),
                    stop=(kt == nkt - 1),
                )

        # software pipeline: S(kt+1) before AV(kt); drip GEMM filler so the
        # PE keeps busy while ACT digests the exps
        emit_scores_exp(0)
        for kt in range(1, nkt):
            emit_scores_exp(kt)
            emit_av(kt - 1)
            drip()
            # emit the previous pair's deferred normalize: for qb>0 the first
            # kts are off-diagonal (no mask-muls on vector to delay), so pop
            # early; for qb==0 every kt is diagonal, pop after those masks
            if kt == (3 if qb == 0 else 1) and pending_normalize:
                pending_normalize.pop()()
        emit_av(nkt - 1)

        # free the AV PSUM banks ASAP with one fast copy each (the next
        # pair's first AV WARs on these banks); normalize runs from SBUF
        # later, off every critical path.  fp16 staging: 2x DVE rate, and
        # raw |O| <~1e4, denom <~3e4 fit fp16 comfortably.
        oraws = []
        for h2 in range(2):
            o_raw = oraw_pool.tile([DP, 512], F16, tag="oraw", name="o_raw")
            eng = nc.scalar.copy if h2 == 0 else nc.vector.tensor_copy
            eng(o_raw[:], ot_ps[h2][:])
            oraws.append(o_raw)

        def normalize():
            if qb == 3 and hp == 3:
                # final pair: its normalize gates the tail projection, so
                # split per-h2 (short pipelined single-partition ops) to
                # minimize chain LATENCY; same tile tags/shapes as below
                recip = recip_pool.tile([1, 2, 512], F32, tag="recip", name="recip")
                bc16 = recip_pool.tile([1, 2, 512], F16, tag="recip16", name="recip16")
                bc = bc_pool.tile([D, 2, 512], F16, tag="bc", name="bc")
                for h2 in range(2):
                    nc.vector.tensor_copy(recip[:, h2, :], oraws[h2][D : D + 1, :])
                    nc.vector.reciprocal_approx_fast(recip[:, h2, :], recip[:, h2, :])
                    nc.vector.tensor_copy(bc16[:, h2, :], recip[:, h2, :])
                    nc.gpsimd.partition_broadcast(bc[:, h2, :], bc16[:, h2, :])
                    nc.vector.tensor_mul(
                        OT[hp][qb][h2 * D : (h2 + 1) * D, :],
                        oraws[h2][:D, :],
                        bc[:, h2, :],
                    )
                return
            # divide rows 0..63 by the sums row (64); both heads' denominator
            # vectors batched into one recip / cast / broadcast
            recip = recip_pool.tile([1, 2, 512], F32, tag="recip", name="recip")
            for h2 in range(2):
                nc.vector.tensor_copy(recip[:, h2, :], oraws[h2][D : D + 1, :])
            nc.vector.reciprocal_approx_fast(recip[:], recip[:])
            bc16 = recip_pool.tile([1, 2, 512], F16, tag="recip16", name="recip16")
            nc.vector.tensor_copy(bc16[:], recip[:])
            bc = bc_pool.tile([D, 2, 512], F16, tag="bc", name="bc")
            nc.gpsimd.partition_broadcast(bc[:], bc16[:])
            for h2 in range(2):
                nc.vector.tensor_mul(
                    OT[hp][qb][h2 * D : (h2 + 1) * D, :],
                    oraws[h2][:D, :],
                    bc[:, h2, :],
                )

        pending_normalize.append(normalize)

    def proj_tile_direct(tt):
        """Tail projection using the (now free) scores PSUM banks."""
        qb, sub = tt // 4, tt % 4
        y_pair_ps = ps_s_pool.tile([P, 2, 512], F32, tag="s_ps", name="y_ps")
        for ct in range(4):
            lhsT = OT[ct][qb][:, sub * P : (sub + 1) * P]
            for nb in range(2):
                nc.tensor.matmul(
                    y_pair_ps[:, nb, :],
                    lhsT=lhsT,
                    rhs=wp_sb[:, ct, nb * 512 : (nb + 1) * 512],
                    start=(ct == 0),
                    stop=(ct == 3),
                )
        if tt % 2 == 0:
            ysbs[tt // 2] = y_pool.tile([P, 2, C], F16, tag="ypair", name="ypair")
        ysb = ysbs[tt // 2]
        for nb in range(2):
            # split engines so the two copies run in parallel at the tail
            eng = nc.scalar.copy if nb == 0 else nc.vector.tensor_copy
            eng(ysb[:, tt % 2, nb * 512 : (nb + 1) * 512], y_pair_ps[:, nb, :])
        if tt >= 14:  # last tiles: DMA singly so the writes start earlier
            nc.sync.dma_start(y[:, tt : tt + 1, :], ysb[:, tt % 2 : tt % 2 + 1, :])
        elif tt % 2 == 1:
            nc.sync.dma_start(y[:, tt - 1 : tt + 1, :], ysb[:])

    # ================= main schedule =================
    flush_until(7)  # all tb=0 K,Q chains: queue PE work spanning DMA arrival
    for qb in range(QB):
        for hp in range(4):
            flush_until(qb * 1000 + hp)  # Q chain for this pair
            attention_pair(qb, hp)
        if qb < 3:
            for tt in range(4 * qb, 4 * qb + 4):
                for nb in range(2):
                    add_proj_chain(qb, tt, nb)
    while pending_normalize:
        pending_normalize.pop()()
    flush_all()
    for tt in range(12, 16):
        proj_tile_direct(tt)

    return nc


_CACHED_NC = None


def get_nc():
    global _CACHED_NC
    if _CACHED_NC is None:
        nc = bacc.Bacc()
        with tile.TileContext(nc) as tc:
            build_attention_kernel(tc)
        nc.compile()
        _CACHED_NC = nc
    return _CACHED_NC


def make_in_maps(x, W_att, W_proj):
    x = np.asarray(x, dtype=np.float32)
    W_att = np.asarray(W_att, dtype=np.float32)
    W_proj = np.asarray(W_proj, dtype=np.float32)
    in_maps = []
    for c in range(NC_CORES):
        b, hg = c // 2, c % 2
        s = hg * 512
        # xh[ci, tb, co, t'] = x[b][tb*512+t', co*128+ci]
        xh = np.ascontiguousarray(
            x[b].reshape(TB, 512, CI, P).transpose(3, 0, 2, 1)
        ).astype(F16_NP)

        def wslice(w):
            # [128(ci), 8(co), 512(m)]
            return np.ascontiguousarray(
                w.reshape(CI, P, 512).transpose(1, 0, 2)
            ).astype(F16_NP)

        in_maps.append(
            {
                "xh": xh,
                "wk": wslice(W_att[:, 0 * C + s : 0 * C + s + 512]),
                "wq": wslice(W_att[:, 1 * C + s : 1 * C + s + 512]),
                "wv": wslice(W_att[:, 2 * C + s : 2 * C + s + 512]),
                "wp": np.ascontiguousarray(
                    W_proj[s : s + 512].reshape(4, P, C).transpose(1, 0, 2)
                ).astype(F16_NP),
            }
        )
    return in_maps


def combine_outputs(results, b_proj):
    B = NC_CORES // 2
    out = np.empty((B, T, C), dtype=np.float32)
    bias = np.asarray(b_proj, dtype=np.float32)
    for b in range(B):
        # y [128(p), 16(tt), 1024] -> [T, C] with t = tt*128 + p
        y0 = np.moveaxis(results[2 * b]["y"].astype(np.float32), 0, 1).reshape(T, C)
        y1 = np.moveaxis(results[2 * b + 1]["y"].astype(np.float32), 0, 1).reshape(T, C)
        out[b] = y0 + y1 + bias
    return out


def kernel(x, W_att, W_proj, b_proj):
    from concourse.bass_utils import run_bass_kernel_spmd

    nc = get_nc()
    in_maps = make_in_maps(x, W_att, W_proj)
    res = run_bass_kernel_spmd(nc, in_maps, list(range(NC_CORES)))
    return combine_outputs(res.results, b_proj)


# revision 65
# speedup vs baseline: 1.0061x; 1.0030x over previous
"""Multi-head causal attention (B=4, T=2048, H=16, D=64) on 8 trn2 NeuronCores.

Sharding: core c = (batch b = c//2, head-group hg = c%2 of 8 heads).
Each core computes its batch's QKV projection for its 8 heads, causal
attention, and a partial output projection (contraction over its 512
channels of W_proj). Host sums the two partials per batch and adds bias.

Schedule: the QKV / output-projection GEMM matmuls are queued as
"filler" steps and dripped between attention score/AV ops so the PE
stays busy while the scalar engine computes the softmax exps (exp
throughput ~956ns per [128,2,512] tile vs ~540ns of attention matmul
per k-tile).  Tag-ordered flushes force each GEMM chain out just
before its first consumer.

Layouts (host-side repacked so every DMA line is 8KB contiguous):
  - xh   [128(ci), 4(tb), 8(co), 512(t)]   x^T tiled
  - wk/wq/wv [128(ci), 8(co), 512(m)]
  - wp   [128(ci), 4(co), 1024(n)]
  - y    [128(p), 16(tt), 1024(n)] fp16 output (partials summed on host)

Per-core kernel layout (as v1):
  - K^T, Q^T stored [hd, t]: head-dim on partitions, 2 heads per tile.
  - V stored [t, h*65+d] with ones column -> AV matmul emits softmax
    denominators in row 64 for free.
  - Scores computed transposed S_T[k, q]; P_T = exp(S_T) is the AV
    moving operand; no max subtraction needed (|scores/8| small).
"""

import os
import sys

import numpy as np

F16_NP = np.dtype(np.float16)

if "/opt/trn_rl_repo" not in sys.path:
    sys.path.insert(0, "/opt/trn_rl_repo")

from collections import deque
from contextlib import ExitStack

import concourse.bass as bass
import concourse.bacc as bacc
import concourse.mybir as mybir
import concourse.tile as tile
from concourse._compat import with_exitstack

P = 128
T = 2048
C = 1024
H_PER_CORE = 8
D = 64
DP = D + 1  # V augmented with a ones column
NC_CORES = 8

TB = 4  # t-blocks of 512
QB = 4  # q-blocks of 512
CI = 8  # contraction tiles of 128 over C for QKV proj

F32 = mybir.dt.float32
F16 = mybir.dt.float16  # full matmul rate, 8x finer mantissa than bf16


@with_exitstack
def build_attention_kernel(ctx: ExitStack, tc: tile.TileContext):
    nc = tc.nc

    xh = nc.declare_dram_parameter("xh", [P, TB, CI, 512], F16, isOutput=False)
    wk = nc.declare_dram_parameter("wk", [P, CI, 512], F16, isOutput=False)
    wq = nc.declare_dram_parameter("wq", [P, CI, 512], F16, isOutput=False)
    wv = nc.declare_dram_parameter("wv", [P, CI, 512], F16, isOutput=False)
    wp = nc.declare_dram_parameter("wp", [P, 4, C], F16, isOutput=False)
    y = nc.declare_dram_parameter("y", [P, 16, C], F16, isOutput=True)

    # ---- SBUF pools ----
    kt_pool = ctx.enter_context(tc.tile_pool(name="ktp", bufs=16))
    qt_pool = ctx.enter_context(tc.tile_pool(name="qtp", bufs=16))
    ot_pool = ctx.enter_context(tc.tile_pool(name="otp", bufs=16))
    v_pool = ctx.enter_context(tc.tile_pool(name="vp", bufs=4))
    const_pool = ctx.enter_context(tc.tile_pool(name="constp", bufs=1))
    w_pool = ctx.enter_context(tc.tile_pool(name="wp_", bufs=1))
    xt_pool = ctx.enter_context(tc.tile_pool(name="xtp", bufs=4))
    pt_pool = ctx.enter_context(tc.tile_pool(name="ptp", bufs=8))
    recip_pool = ctx.enter_context(tc.tile_pool(name="recipp", bufs=4))
    bc_pool = ctx.enter_context(tc.tile_pool(name="bcp", bufs=4))
    oraw_pool = ctx.enter_context(tc.tile_pool(name="orawp", bufs=4))
    y_pool = ctx.enter_context(tc.tile_pool(name="yp", bufs=2))
    # ---- PSUM: 4 banks scores + 2 banks AV + 2 banks GEMM filler = 8 ----
    ps_s_pool = ctx.enter_context(tc.tile_pool(name="ps_s", bufs=2, space="PSUM"))
    ps_o_pool = ctx.enter_context(tc.tile_pool(name="ps_o", bufs=2, space="PSUM"))
    ps_f_pool = ctx.enter_context(tc.tile_pool(name="ps_f", bufs=2, space="PSUM"))

    # KT[pt][tb], QT[pt][qb]: [128, 512]; partitions = 2 heads x 64 dims
    KT = [[kt_pool.tile([P, 512], F16, tag="kt", name=f"KT_{pt}_{tb}") for tb in range(TB)] for pt in range(4)]
    QT = [[qt_pool.tile([P, 512], F16, tag="qt", name=f"QT_{pt}_{qb}") for qb in range(QB)] for pt in range(4)]
    OT = [[ot_pool.tile([P, 512], F16, tag="ot", name=f"OT_{hp}_{qb}") for qb in range(QB)] for hp in range(4)]
    V = [v_pool.tile([P, 4, H_PER_CORE * DP], F16, tag="v", name=f"V_{tb}") for tb in range(TB)]
    masks = const_pool.tile([P, 4, 512], F16, tag="masks", name="masks")
    wk_sb = w_pool.tile([P, CI, 512], F16)
    wq_sb = w_pool.tile([P, CI, 512], F16)
    wv_sb = w_pool.tile([P, CI, 512], F16)
    wp_sb = w_pool.tile([P, 4, C], F16)
    xts = [xt_pool.tile([P, CI, 512], F16, tag="xt", name=f"xt_{tb}") for tb in range(TB)]

    # diagonal causal masks: masks[:, j, :][kk, qq] = 1.0 if qq >= kk + j*128
    for j in range(4):
        nc.gpsimd.memset(masks[:, j, :], 1.0)
        nc.gpsimd.affine_select(
            out=masks[:, j, :],
            in_=masks[:, j, :],
            compare_op=mybir.AluOpType.is_ge,
            fill=0.0,
            base=-j * P,
            pattern=[[1, 512]],
            channel_multiplier=-1,
        )
    # ones column of V
    for tb in range(TB):
        ones_col = V[tb].rearrange("p s (h e) -> p s h e", e=DP)[:, :, :, D : D + 1]
        nc.gpsimd.memset(ones_col, 1.0)

    # ---- DMAs: wq / x chunks first (Q chains run first) so the PE starts
    # ASAP; then prefetch everything (all lines 8KB contiguous). ----
    nc.sync.dma_start(wq_sb[:, 0:1], wq[:, 0:1])
    nc.sync.dma_start(xts[0][:, 0:1], xh[:, 0, 0:1])
    nc.sync.dma_start(wq_sb[:, 1:2], wq[:, 1:2])
    nc.sync.dma_start(xts[0][:, 1:2], xh[:, 0, 1:2])
    nc.sync.dma_start(wq_sb[:, 2:3], wq[:, 2:3])
    nc.sync.dma_start(xts[0][:, 2:3], xh[:, 0, 2:3])
    nc.sync.dma_start(wq_sb[:, 3:], wq[:, 3:])
    nc.sync.dma_start(xts[0][:, 3:5], xh[:, 0, 3:5])
    nc.sync.dma_start(xts[0][:, 5:8], xh[:, 0, 5:8])
    nc.sync.dma_start(wk_sb[:], wk[:])
    nc.sync.dma_start(wv_sb[:], wv[:])
    nc.sync.dma_start(xts[1][:], xh[:, 1])
    nc.sync.dma_start(xts[2][:], xh[:, 2])
    nc.sync.dma_start(xts[3][:], xh[:, 3])
    nc.sync.dma_start(wp_sb[:], wp[:])

    # ================= filler machinery =================
    # Each filler item: (tag, fn). Tags are appended nondecreasing.
    # qkv chain tags: tb*1000 + {Q0..Q3: 0..3, K0..K3: 4..7, V0..V3: 8..11}
    # (Q flushes at pair start; K lazily at the first diagonal score; V at
    # the diagonal AVs — spreads forced chains across each pair.)
    # proj tags: 10000 + qb*10
    filler = deque()
    drip_clock = [0]

    def drip():
        # 2,2,1 pattern ~= the per-kt PE deficit (exp 956ns vs ~540ns of
        # attention matmul); rations filler so it lasts to the final pair
        n = 1 if drip_clock[0] % 3 == 2 else 2
        drip_clock[0] += 1
        while n > 0 and filler:
            _, fn = filler.popleft()
            fn()
            n -= 1

    def flush_until(tag_limit):
        while filler and filler[0][0] <= tag_limit:
            _, fn = filler.popleft()
            fn()

    def flush_all():
        while filler:
            _, fn = filler.popleft()
            fn()

    def add_kq_chain(tb, kind, pt):
        """K^T / Q^T chain: out[hd, t] for 128 hd (2 heads), 512 t."""
        w_sb = wk_sb if kind == "K" else wq_sb
        dst = KT if kind == "K" else QT
        tag = tb * 1000 + pt + (4 if kind == "K" else 0)
        ps_ref = []

        def mk(ci):
            def fn():
                if ci == 0:
                    ps_ref.append(ps_f_pool.tile([P, 512], F32, tag="f_ps", name="f_ps"))
                nc.tensor.matmul(
                    ps_ref[0][:],
                    lhsT=w_sb[:, ci, pt * P : (pt + 1) * P],
                    rhs=xts[tb][:, ci, :],
                    start=(ci == 0),
                    stop=(ci == CI - 1),
                )
                if ci == CI - 1:
                    nc.vector.tensor_copy(dst[pt][tb][:], ps_ref[0][:])

            return fn

        for ci in range(CI):
            filler.append((tag, mk(ci)))

    def add_v_chain(tb, ts):
        """V chain: out[t-slice 128, h*d 512] scattered into V[tb] layout."""
        tag = tb * 1000 + 8 + ts
        ps_ref = []

        def mk(ci):
            def fn():
                if ci == 0:
                    ps_ref.append(ps_f_pool.tile([P, 512], F32, tag="f_ps", name="f_ps"))
                nc.tensor.matmul(
                    ps_ref[0][:],
                    lhsT=xts[tb][:, ci, ts * P : (ts + 1) * P],
                    rhs=wv_sb[:, ci, :],
                    start=(ci == 0),
                    stop=(ci == CI - 1),
                )
                if ci == CI - 1:
                    nc.vector.tensor_copy(
                        V[tb][:, ts].rearrange("p (h e) -> p h e", e=DP)[:, :, :D],
                        ps_ref[0][:].rearrange("p (h d) -> p h d", d=D),
                    )

            return fn

        for ci in range(CI):
            filler.append((tag, mk(ci)))

    ysbs = {}

    def add_proj_chain(qb, tt, nb):
        """Output projection partial: y[t-tile, 512 nb-cols] = sum_ct OT."""
        tag = 10000 + qb * 10
        sub = tt % 4
        ps_ref = []

        def mk(ct):
            def fn():
                if ct == 0:
                    ps_ref.append(ps_f_pool.tile([P, 512], F32, tag="f_ps", name="f_ps"))
                nc.tensor.matmul(
                    ps_ref[0][:],
                    lhsT=OT[ct][qb][:, sub * P : (sub + 1) * P],
                    rhs=wp_sb[:, ct, nb * 512 : (nb + 1) * 512],
                    start=(ct == 0),
                    stop=(ct == 3),
                )
                if ct == 3:
                    if tt % 2 == 0 and nb == 0:
                        ysbs[tt // 2] = y_pool.tile([P, 2, C], F16, tag="ypair", name="ypair")
                    ysb = ysbs[tt // 2]
                    nc.vector.tensor_copy(
                        ysb[:, tt % 2, nb * 512 : (nb + 1) * 512], ps_ref[0][:]
                    )
                    if tt % 2 == 1 and nb == 1:
                        nc.sync.dma_start(y[:, tt - 1 : tt + 1, :], ysb[:])

            return fn

        for ct in range(4):
            filler.append((tag, mk(ct)))

    for tb in range(TB):
        for pt in range(4):
            add_kq_chain(tb, "Q", pt)
        for pt in range(4):
            add_kq_chain(tb, "K", pt)
        for ts in range(4):
            add_v_chain(tb, ts)

    # ================= attention =================
    pending_normalize = []
    bcs = []

    def attention_pair(qb, hp):
        ot_ps = [ps_o_pool.tile([DP, 512], F32, tag="ot_ps", name=f"ot_ps_{i}") for i in range(2)]
        nkt = 4 * (qb + 1)
        pts = {}

        def emit_scores_exp(kt):
            tb = kt // 4
            if kt >= 4 * qb:  # diagonal score needs this qb's K chain
                flush_until(qb * 1000 + 4 + hp)
            # diagonal tiles: only q >= j*128 is (partially) visible
            qs = (kt - 4 * qb) * P if kt >= 4 * qb else 0
            nq = 512 - qs
            s_ps = ps_s_pool.tile([P, 2, 512], F32, tag="s_ps", name="s_ps")
            for h2 in range(2):
                # S_T[k, q] for head h = 2*hp + h2 (row-packed pair)
                nc.tensor.matmul(
                    s_ps[:, h2, qs:],
                    lhsT=KT[hp][tb][
                        h2 * D : (h2 + 1) * D,
                        (kt % 4) * P : (kt % 4 + 1) * P,
                    ],
                    rhs=QT[hp][qb][h2 * D : (h2 + 1) * D, qs:],
                    start=True,
                    stop=True,
                )
            p_t = pt_pool.tile([P, 2, 512], F16, tag="pt", name="p_t")
            nc.scalar.activation(
                p_t[:, :, qs:],
                s_ps[:, :, qs:],
                mybir.ActivationFunctionType.Exp,
                scale=0.125,
            )
            if kt >= 4 * qb:  # diagonal: zero q < k entries.  Only the first
                # 128 q-columns of the tile can be masked (q >= k holds for
                # all k once q passes the k-tile) -> 1/4 the mask-mul work.
                j = kt - 4 * qb
                mb = masks[:, j : j + 1, qs : qs + P].to_broadcast([P, 2, P])
                nc.vector.tensor_mul(
                    p_t[:, :, qs : qs + P], p_t[:, :, qs : qs + P], mb
                )
            pts[kt] = (p_t, qs)

        def emit_av(kt):
            tb = kt // 4
            if kt >= 4 * qb:  # diagonal AV needs V[qb] chain ts = kt-4qb
                flush_until(qb * 1000 + 8 + (kt - 4 * qb))
            p_t, qs = pts.pop(kt)
            for h2 in range(2):
                h = 2 * hp + h2
                nc.tensor.matmul(
                    ot_ps[h2][:, qs:],
                    lhsT=V[tb][:, kt % 4, h * DP : (h + 1) * DP],
                    rhs=p_t[:, h2, qs:],
                    start=(kt == 0),
                    stop=(kt == nkt - 1),
                )

        # software pipeline: S(kt+1) before AV(kt); drip GEMM filler so the
        # PE keeps busy while ACT digests the exps
        emit_scores_exp(0)
        for kt in range(1, nkt):
            emit_scores_exp(kt)
            emit_av(kt - 1)
            drip()
            # emit the previous pair's deferred normalize: for qb>0 the first
            # kts are off-diagonal (no mask-muls on vector to delay), so pop
            # early; for qb==0 every kt is diagonal, pop after those masks
            if kt == (3 if qb == 0 else 1) and pending_normalize:
                pending_normalize.pop(0)()
        emit_av(nkt - 1)

        # free the AV PSUM banks ASAP with one fast copy each (the next
        # pair's first AV WARs on these banks); normalize runs from SBUF
        # later, off every critical path.  fp16 staging: 2x DVE rate, and
        # raw |O| <~1e4, denom <~3e4 fit fp16 comfortably.
        oraws = []
        for h2 in range(2):
            o_raw = oraw_pool.tile([DP, 512], F16, tag="oraw", name="o_raw")
            eng = nc.scalar.copy if h2 == 0 else nc.vector.tensor_copy
            eng(o_raw[:], ot_ps[h2][:])
            oraws.append(o_raw)

        def normalize():
            if qb == 3 and hp == 3:
                # final pair: its normalize gates the tail projection, so
                # split per-h2 (short pipelined single-partition ops) to
                # minimize chain LATENCY; same tile tags/shapes as below
                recip = recip_pool.tile([1, 2, 512], F32, tag="recip", name="recip")
                bc16 = recip_pool.tile([1, 2, 512], F16, tag="recip16", name="recip16")
                bc = bc_pool.tile([D, 2, 512], F16, tag="bc", name="bc")
                for h2 in range(2):
                    nc.vector.tensor_copy(recip[:, h2, :], oraws[h2][D : D + 1, :])
                    nc.vector.reciprocal_approx_fast(recip[:, h2, :], recip[:, h2, :])
                    nc.vector.tensor_copy(bc16[:, h2, :], recip[:, h2, :])
                    nc.gpsimd.partition_broadcast(bc[:, h2, :], bc16[:, h2, :])
                    nc.vector.tensor_mul(
                        OT[hp][qb][h2 * D : (h2 + 1) * D, :],
                        oraws[h2][:D, :],
                        bc[:, h2, :],
                    )
                return
            # divide rows 0..63 by the sums row (64); both heads' denominator
            # vectors batched into one recip / cast / broadcast
            recip = recip_pool.tile([1, 2, 512], F32, tag="recip", name="recip")
            for h2 in range(2):
                nc.vector.tensor_copy(recip[:, h2, :], oraws[h2][D : D + 1, :])
            nc.vector.reciprocal_approx_fast(recip[:], recip[:])
            bc16 = recip_pool.tile([1, 2, 512], F16, tag="recip16", name="recip16")
            nc.vector.tensor_copy(bc16[:], recip[:])
            bc = bc_pool.tile([D, 2, 512], F16, tag="bc", name="bc")
            nc.gpsimd.partition_broadcast(bc[:], bc16[:])
            for h2 in range(2):
                nc.vector.tensor_mul(
                    OT[hp][qb][h2 * D : (h2 + 1) * D, :],
                    oraws[h2][:D, :],
                    bc[:, h2, :],
                )

        pending_normalize.append(normalize)

    def proj_tile_direct(tt):
        """Tail projection using the (now free) scores PSUM banks."""
        qb, sub = tt // 4, tt % 4
        y_pair_ps = ps_s_pool.tile([P, 2, 512], F32, tag="s_ps", name="y_ps")
        for ct in range(4):
            lhsT = OT[ct][qb][:, sub * P : (sub + 1) * P]
            for nb in range(2):
                nc.tensor.matmul(
                    y_pair_ps[:, nb, :],
                    lhsT=lhsT,
                    rhs=wp_sb[:, ct, nb * 512 : (nb + 1) * 512],
                    start=(ct == 0),
                    stop=(ct == 3),
                )
        if tt % 2 == 0:
            ysbs[tt // 2] = y_pool.tile([P, 2, C], F16, tag="ypair", name="ypair")
        ysb = ysbs[tt // 2]
        for nb in range(2):
            # split engines so the two copies run in parallel at the tail
            eng = nc.scalar.copy if nb == 0 else nc.vector.tensor_copy
            eng(ysb[:, tt % 2, nb * 512 : (nb + 1) * 512], y_pair_ps[:, nb, :])
        if tt >= 14:  # last tiles: DMA singly so the writes start earlier
            nc.sync.dma_start(y[:, tt : tt + 1, :], ysb[:, tt % 2 : tt % 2 + 1, :])
        elif tt % 2 == 1:
            nc.sync.dma_start(y[:, tt - 1 : tt + 1, :], ysb[:])

    # ================= main schedule =================
    flush_until(7)  # all tb=0 K,Q chains: queue PE work spanning DMA arrival
    for qb in range(QB):
        for hp in range(4):
            flush_until(qb * 1000 + hp)  # Q chain for this pair
            attention_pair(qb, hp)
        if qb < 3:
            for tt in range(4 * qb, 4 * qb + 4):
                for nb in range(2):
                    add_proj_chain(qb, tt, nb)
    while pending_normalize:
        pending_normalize.pop()()
    flush_all()
    for tt in range(12, 16):
        proj_tile_direct(tt)

    return nc


_CACHED_NC = None


def get_nc():
    global _CACHED_NC
    if _CACHED_NC is None:
        nc = bacc.Bacc()
        with tile.TileContext(nc) as tc:
            build_attention_kernel(tc)
        nc.compile()
        _CACHED_NC = nc
    return _CACHED_NC


def make_in_maps(x, W_att, W_proj):
    x = np.asarray(x, dtype=np.float32)
    W_att = np.asarray(W_att, dtype=np.float32)
    W_proj = np.asarray(W_proj, dtype=np.float32)
    in_maps = []
    for c in range(NC_CORES):
        b, hg = c // 2, c % 2
        s = hg * 512
        # xh[ci, tb, co, t'] = x[b][tb*512+t', co*128+ci]
        xh = np.ascontiguousarray(
            x[b].reshape(TB, 512, CI, P).transpose(3, 0, 2, 1)
        ).astype(F16_NP)

        def wslice(w):
            # [128(ci), 8(co), 512(m)]
            return np.ascontiguousarray(
                w.reshape(CI, P, 512).transpose(1, 0, 2)
            ).astype(F16_NP)

        in_maps.append(
            {
                "xh": xh,
                "wk": wslice(W_att[:, 0 * C + s : 0 * C + s + 512]),
                "wq": wslice(W_att[:, 1 * C + s : 1 * C + s + 512]),
                "wv": wslice(W_att[:, 2 * C + s : 2 * C + s + 512]),
                "wp": np.ascontiguousarray(
                    W_proj[s : s + 512].reshape(4, P, C).transpose(1, 0, 2)
                ).astype(F16_NP),
            }
        )
    return in_maps


def combine_outputs(results, b_proj):
    B = NC_CORES // 2
    out = np.empty((B, T, C), dtype=np.float32)
    bias = np.asarray(b_proj, dtype=np.float32)
    for b in range(B):
        # y [128(p), 16(tt), 1024] -> [T, C] with t = tt*128 + p
        y0 = np.moveaxis(results[2 * b]["y"].astype(np.float32), 0, 1).reshape(T, C)
        y1 = np.moveaxis(results[2 * b + 1]["y"].astype(np.float32), 0, 1).reshape(T, C)
        out[b] = y0 + y1 + bias
    return out


def kernel(x, W_att, W_proj, b_proj):
    from concourse.bass_utils import run_bass_kernel_spmd

    nc = get_nc()
    in_maps = make_in_maps(x, W_att, W_proj)
    res = run_bass_kernel_spmd(nc, in_maps, list(range(NC_CORES)))
    return combine_outputs(res.results, b_proj)
